# revision 36
# baseline (speedup 1.0000x reference)
"""MoE actor kernel for 8 TRN2 NeuronCores (expert-parallel, host routing).

Problem: B=65536 tokens, obs dim D=376, each routed by `o` to one of E=8
experts; per-expert MLP 376 -> 256 -> 256 -> {mean[17], log_std[17]} with
relu/relu/(identity|tanh-affine) heads.

Strategy: routing/gather happens on the host (numpy) — core e receives
exactly the tokens assigned to expert e (padded to full 512-token tiles
plus one short remainder tile) and only that expert's weights. Every core
runs the same dense 3-layer MLP graph with features on the partition axis:

    h1T[H, n] = relu(W1.T @ xT + b1)     K=384(pad of 376) -> M=256
    h2T[H, n] = relu(W2.T @ h1T + b2)    K=256 -> M=256
    zT[128, n] = Wc.T @ h2T              K=256 -> M=128 (mean @0:17, z @32:49)

Matmuls run in bf16 (full PE rate, FWL weight loads; ~4e-3 rel err, fp32
PSUM accumulate). L3+epilogue are deferred one tile; the critical h1 ReLUs
sit on ScalarE (h1[1]) and VectorE (h1[0]) so the PE's residual per-tile
wait is absorbed as a single stretched matmul, NOT an idle gap — packed
100%-duty schedules latch the chip into the P0 power state (~2.0 GHz),
while this form sustains 2.4 GHz. A VectorE-memset-gated warm chain opens
the HAM clock window (1.2 -> 2.4 GHz after ~3.4us sustained busy) from
engine bring-up. Mean and raw-z rows leave the chip bf16 via one wide
ScalarE copy + tile-paired DMAs on spread queues; the host applies bm and
3.5*tanh(z + bs) - 1.5 in f32 during the scatter.
"""

import numpy as np

B, D, H, A, E = 65536, 376, 256, 17, 8
DPAD = 384          # D padded to 3 partition tiles of 128
TOK = 512           # token tile (matmul free dim; one PSUM bank)
AOUT = 2 * A        # 34: mean ++ log_std

# test.py hooks: set TRACE=True before calling kernel() to profile; the
# BassKernelResults of the last run lands in LAST_RESULT.
TRACE = False
TRACE_CORES = None
LAST_RESULT = None

_cache = {}


def _install_axon_ntff_hook():
    """antenv.axon_hooks is absent in this image; recreate it so
    run_bass_kernel_spmd(trace=True) can capture NTFF profiles."""
    import sys, types
    if 'antenv.axon_hooks' in sys.modules:
        return
    try:
        from trn_agent_boot.trn_boot import _ntff_profile_via_ctypes
        hook = _ntff_profile_via_ctypes('/opt/axon/libaxon_pjrt.so')
    except Exception:
        hook = None
    m = types.ModuleType('antenv.axon_hooks')
    m.get_axon_ntff_profile_hook = lambda: hook
    m.set_axon_ntff_profile_hook = lambda h: None
    sys.modules['antenv.axon_hooks'] = m


def _build(n_full, rem):
    import concourse.bass as bass
    import concourse.tile as tile
    from concourse import bacc, mybir

    f32 = mybir.dt.float32
    bf16 = mybir.dt.bfloat16
    AF = mybir.ActivationFunctionType
    ds = bass.ds
    sizes = [TOK] * n_full + ([rem] if rem else [])
    npad = n_full * TOK + rem
    T = len(sizes)
    offs = [0] + list(np.cumsum(sizes))[:-1]

    nc = bacc.Bacc("TRN2", target_bir_lowering=False, debug=False, num_devices=E)
    x_ext = nc.dram_tensor("x", [128, 3 * npad], bf16, kind="ExternalInput")
    wt_ext = nc.dram_tensor("wt", [128, 1544], bf16, kind="ExternalInput")
    out_ext = nc.dram_tensor("out", [AOUT, npad], bf16, kind="ExternalOutput")

    with tile.TileContext(nc) as tc:
        with tc.tile_pool(name="wp", bufs=1) as wp, \
             tc.tile_pool(name="xp", bufs=4) as xp, \
             tc.tile_pool(name="hp", bufs=3) as hp, \
             tc.tile_pool(name="op", bufs=4) as op, \
             tc.tile_pool(name="ps1", bufs=1, space="PSUM") as ps1, \
             tc.tile_pool(name="ps2", bufs=1, space="PSUM") as ps2, \
             tc.tile_pool(name="ps3", bufs=2, space="PSUM") as ps3:
            wts = wp.tile([128, 1544], bf16)
            bias = wp.tile([128, 5], f32)
            w1 = wts[:, ds(0, 3 * H)]
            w2 = wts[:, ds(3 * H, 2 * H)]
            wc = wts[:, ds(5 * H, 2 * 128)]
            b1 = bias[:, ds(0, 2)]
            b2 = bias[:, ds(2, 2)]
            bc = bias[:, ds(4, 1)]

            # Prologue. DMA cost model: ~0.7us of descriptor-gen on the
            # issuing queue per dma_start, then ~1 descriptor per SBUF
            # partition serviced FIFO across the 16 rings (so a [128, c]
            # transfer costs 128 descriptors no matter how small c is).
            # All weights+biases travel as ONE [128, 1544] bf16 transfer;
            # x tile 0 is split into its three k-chunks so the k-major
            # L1(0) can start on chunk 0 while chunks 1-2 stream in.
            nc.scalar.dma_start(wts[:], wt_ext.ap()[:])
            xsb = [None] * T
            n0 = sizes[0]
            xsb[0] = xp.tile([128, 3 * TOK], bf16, tag="x", name="xsb")
            nc.sync.dma_start(xsb[0][:, 0:3 * n0], x_ext.ap()[:, 0:3 * n0])
            # biases are consumed as f32 APs: one 5-column cast, done long
            # before the first ReLU needs it.
            # Memset-gated PE warm-up chain: the memset is VectorE's first
            # instruction (no DMA dependency) so the chain starts right at
            # engine bring-up, opening the free-running HAM activity window
            # (3.4us sustained busy -> 2.4 GHz) as early as possible.
            warm = wp.tile([128, TOK], bf16, name="warm")
            nc.vector.memset(warm[:], 0.0)
            pwarm = ps2.tile([128, TOK], f32, tag="p2_0", name="p2_0")
            for _ in range(12):
                nc.tensor.matmul(pwarm[:, 0:TOK], warm[:, ds(0, 128)],
                                 warm[:, 0:TOK], start=True, stop=True)
            # biases are consumed as f32 APs: one 5-column cast, done long
            # before the first ReLU needs it.
            nc.vector.tensor_copy(bias[:], wts[:, ds(1536, 5)])
            # x(1)/x(2) are deliberately NOT issued here: their ring
            # descriptors would interleave with wt/x(0) and delay the first
            # L1 matmul by ~3us. The loop issues x(t+1) and x(t+2) at t=0.

            def filler(nf):
                # Garbage matmuls reading only the (resident) weights tile.
                # They keep the PE busy across the t=0/t=1 ReLU latency
                # bubbles: any PE idle gap in the first ~2 tiles resets the
                # HAM activity window and delays the 1.2->2.4 GHz unthrottle.
                for _ in range(nf):
                    pf = ps3.tile([128, TOK], f32, tag="p3", name="p3")
                    nc.tensor.matmul(pf[:, 0:TOK], wts[:, ds(0, 128)],
                                     wts[:, ds(0, TOK)], start=True, stop=True)

            # Epilogue state: consecutive tiles pair into one [64, 2*TOK]
            # SBUF tile so each out-DMA trigger (~0.8us of descriptor-gen
            # on its queue) covers two tiles. Activation partition bases
            # must be 32-aligned, so tanh lands at rows 32:49 and the DMA
            # ships rows 0:17 and 32:49 separately.
            pair = [None, 0]          # [tile handle, start tile index]

            def head_tail(t, h2, last=False):
                # L3 + epilogue for tile t (deferred one iteration so the
                # PE rolls straight into the next tile's L1/L2). Mean rows
                # 0:17 and raw z rows 32:49 leave PSUM bf16; the host adds
                # bm and applies 3.5*tanh(z + bs) - 1.5 in f32.
                n = sizes[t]
                p3 = ps3.tile([128, TOK], f32, tag="p3", name="p3")
                for k in range(2):
                    nc.tensor.matmul(
                        p3[:, 0:n], wc[:, ds(k * 128, 128)], h2[k][:, 0:n],
                        start=(k == 0), stop=(k == 1))
                if pair[0] is None:
                    pair[0] = op.tile([64, 2 * TOK], bf16, tag="ot",
                                      name="ot")
                    pair[1] = t
                ot = pair[0]
                c0 = offs[t] - offs[pair[1]]
                # One wide cast covers mean rows 0:17 AND raw z rows 32:49
                # (rows 17:32 are zeros from wc's zero columns). tanh+bias
                # live on the host. Mid-run the cast runs on ScalarE (which
                # has ~1.2us/tile of slack) so VectorE's queue drains well
                # before the next tile's h1[0] gate — its backlog margin
                # was only ~117ns with the cast on VectorE. The final tile
                # keeps it on VectorE so the drain-chain triggers don't
                # serialize behind ScalarE.
                if last:
                    nc.vector.tensor_copy(ot[0:49, c0:c0 + n], p3[0:49, 0:n])
                else:
                    nc.scalar.activation(ot[0:49, c0:c0 + n], p3[0:49, 0:n],
                                         AF.Copy)
                if t > pair[1] or last:
                    off = offs[pair[1]]
                    w = c0 + n
                    if last:
                        # Final flush: split the two triggers across the
                        # scalar and vector queues so their descriptor-gen
                        # runs in parallel right after tanh/copy, instead
                        # of serializing behind the gpsimd queue.
                        nc.sync.dma_start(
                            out_ext.ap()[0:A, off:off + w], ot[0:A, 0:w])
                        nc.scalar.dma_start(
                            out_ext.ap()[A:AOUT, off:off + w],
                            ot[32:32 + A, 0:w])
                    else:
                        # Split across two queues: halves each queue's
                        # trigger load and spreads the output descriptors
                        # over two ring sets, shortening the final drain.
                        nc.gpsimd.dma_start(
                            out_ext.ap()[0:A, off:off + w], ot[0:A, 0:w])
                        nc.sync.dma_start(
                            out_ext.ap()[A:AOUT, off:off + w],
                            ot[32:32 + A, 0:w])
                    pair[0] = None

            prev = None
            for t, n in enumerate(sizes):
                pre = [t + 1, t + 2] if t == 0 else [t + 2]
                for tp in pre:
                    if tp < T and xsb[tp] is None:
                        xsb[tp] = xp.tile([128, 3 * TOK], bf16, tag="x",
                                          name="xsb")
                        xoff = 3 * offs[tp]
                        nc.sync.dma_start(xsb[tp][:, 0:3 * sizes[tp]],
                                          x_ext.ap()[:, xoff:
                                                      xoff + 3 * sizes[tp]])
                xk = [xsb[t][:, ds(k * n, n)] for k in range(3)]

                p1 = [ps1.tile([128, TOK], f32, tag=f"p1_{m}", name=f"p1_{m}")
                      for m in range(2)]
                if t == 0:
                    km_order = [(k, m) for k in range(3) for m in range(2)]
                else:
                    km_order = [(k, m) for m in range(2) for k in range(3)]
                for k, m in km_order:
                    nc.tensor.matmul(
                        p1[m][:, 0:n], w1[:, ds(k * H + m * 128, 128)],
                        xk[k], start=(k == 0), stop=(k == 2))
                h1 = []
                for m in range(2):
                    h = hp.tile([128, TOK], bf16, tag=f"h1_{m}",
                                name=f"h1_{m}")
                    if t == T - 1 and t > 0:
                        # Last tile: both queues go idle afterwards, so
                        # split each ReLU across ScalarE+VectorE to halve
                        # the unhideable end-of-pipeline latency.
                        hn = n // 2
                        nc.scalar.activation(h[:, 0:hn], p1[m][:, 0:hn],
                                             AF.Relu, bias=b1[:, ds(m, 1)])
                        nc.vector.tensor_scalar(
                            out=h[:, hn:n], in0=p1[m][:, hn:n],
                            scalar1=b1[:, ds(m, 1)], scalar2=0.0,
                            op0=mybir.AluOpType.add, op1=mybir.AluOpType.max)
                    elif m == 0:
                        # h1[0] gates L2 k=0 (early need, ~430ns more slack)
                        # -> VectorE; h1[1] gates L2 k=1 on the critical
                        # path -> ScalarE's activation is ~60ns faster.
                        nc.vector.tensor_scalar(
                            out=h[:, 0:n], in0=p1[m][:, 0:n],
                            scalar1=b1[:, ds(m, 1)], scalar2=0.0,
                            op0=mybir.AluOpType.add, op1=mybir.AluOpType.max)
                    else:
                        nc.scalar.activation(h[:, 0:n], p1[m][:, 0:n], AF.Relu,
                                             bias=b1[:, ds(m, 1)])
                    h1.append(h)

                if t == 0:
                    # Partial filler for the t=0 L2 bubble (the h1 ReLU
                    # latency is unhidden on the very first tile): N=384
                    # leaves a ~0.3us gap — small enough not to reset the
                    # HAM window, large enough to stay off the P0 power
                    # throttle that a 100%-duty PE triggers.
                    for nf in (TOK, 320):
                        pf = ps3.tile([128, TOK], f32, tag="p3", name="p3")
                        nc.tensor.matmul(pf[:, 0:nf], wts[:, ds(0, 128)],
                                         wts[:, 0:nf], start=True, stop=True)

                if prev is not None:
                    head_tail(prev[0], prev[1])

                # k-major order: the k=0 matmuls only need h1[0], giving the
                # engine producing h1[1] time to finish.
                p2 = [ps2.tile([128, TOK], f32, tag=f"p2_{m}", name=f"p2_{m}")
                      for m in range(2)]
                for k in range(2):
                    for m in range(2):
                        nc.tensor.matmul(
                            p2[m][:, 0:n], w2[:, ds(k * H + m * 128, 128)],
                            h1[k][:, 0:n],
                            start=(k == 0), stop=(k == 1))
                h2 = []
                for m in range(2):
                    h = hp.tile([128, TOK], bf16, tag=f"h2_{m}",
                                name=f"h2_{m}")
                    if t == T - 1 and t > 0:
                        hn = n // 2
                        nc.scalar.activation(h[:, 0:hn], p2[m][:, 0:hn],
                                             AF.Relu, bias=b2[:, ds(m, 1)])
                        nc.vector.tensor_scalar(
                            out=h[:, hn:n], in0=p2[m][:, hn:n],
                            scalar1=b2[:, ds(m, 1)], scalar2=0.0,
                            op0=mybir.AluOpType.add, op1=mybir.AluOpType.max)
                    elif m == 0:
                        nc.scalar.activation(h[:, 0:n], p2[m][:, 0:n], AF.Relu,
                                             bias=b2[:, ds(m, 1)])
                    else:
                        nc.vector.tensor_scalar(
                            out=h[:, 0:n], in0=p2[m][:, 0:n],
                            scalar1=b2[:, ds(m, 1)], scalar2=0.0,
                            op0=mybir.AluOpType.add, op1=mybir.AluOpType.max)
                    h2.append(h)

                prev = (t, h2)
            head_tail(prev[0], prev[1], last=True)

    nc.compile()
    return nc


def _get_compiled(n_full, rem):
    key = (n_full, rem)
    nc = _cache.get(key)
    if nc is None:
        nc = _build(n_full, rem)
        _cache[key] = nc
    return nc


def kernel(x, o, W1, b1, W2, b2, Wm, bm, Ws, bs):
    global LAST_RESULT
    import ml_dtypes
    from concourse import bass_utils

    x = np.asarray(x, dtype=np.float32)
    o_i = np.asarray(o).astype(np.int64)
    W1 = np.asarray(W1, dtype=np.float32)
    b1 = np.asarray(b1, dtype=np.float32)
    W2 = np.asarray(W2, dtype=np.float32)
    b2 = np.asarray(b2, dtype=np.float32)
    Wm = np.asarray(Wm, dtype=np.float32)
    bm = np.asarray(bm, dtype=np.float32)
    Ws = np.asarray(Ws, dtype=np.float32)
    bs = np.asarray(bs, dtype=np.float32)

    nb, d = x.shape
    counts = np.bincount(o_i, minlength=E)
    cmax = int(counts.max())
    n_full = max(1, cmax // TOK)
    rem = -(-max(0, cmax - n_full * TOK) // 128) * 128
    npad = n_full * TOK + rem
    order = np.argsort(o_i, kind="stable")
    idx_per_e = np.split(order, np.cumsum(counts)[:-1])
    sizes = [TOK] * n_full + ([rem] if rem else [])
    offs = [0] + list(np.cumsum(sizes))[:-1]

    in_maps = []
    for e in range(E):
        idx = idx_per_e[e]
        xg = np.zeros((npad, DPAD), ml_dtypes.bfloat16)
        xg[:len(idx), :d] = x[idx].astype(ml_dtypes.bfloat16)
        x_pack = np.concatenate(
            [xg[off:off + n].reshape(n, 3, 128).transpose(2, 1, 0).reshape(
                128, 3 * n) for off, n in zip(offs, sizes)], axis=1)
        x_pack = np.ascontiguousarray(x_pack)

        w1p = np.zeros((DPAD, H), np.float32)
        w1p[:d] = W1[e]
        w1_pack = np.ascontiguousarray(
            w1p.reshape(3, 128, H).transpose(1, 0, 2)).reshape(128, 3 * H)
        w2_pack = np.ascontiguousarray(
            W2[e].reshape(2, 128, H).transpose(1, 0, 2)).reshape(128, 2 * H)
        wc_full = np.zeros((H, 128), np.float32)
        wc_full[:, 0:A] = Wm[e]
        wc_full[:, 32:32 + A] = Ws[e]
        wc_pack = np.ascontiguousarray(
            wc_full.reshape(2, 128, 128).transpose(1, 0, 2)).reshape(
                128, 2 * 128)
        b1_pack = np.ascontiguousarray(b1[e].reshape(2, 128).T)
        b2_pack = np.ascontiguousarray(b2[e].reshape(2, 128).T)
        bc_pack = np.zeros((128, 1), np.float32)
        bc_pack[32:32 + A, 0] = bs[e]
        pad = np.zeros((128, 3), np.float32)
        wt_pack = np.concatenate(
            [w1_pack, w2_pack, wc_pack, b1_pack, b2_pack, bc_pack, pad],
            axis=1).astype(ml_dtypes.bfloat16)

        in_maps.append({"x": x_pack, "wt": wt_pack})

    nc = _get_compiled(n_full, rem)

    kwargs = {}
    if TRACE:
        _install_axon_ntff_hook()
        bass_utils.upload_artifacts = lambda tmpdir: f"local:{tmpdir}"
        kwargs["trace"] = True
        if TRACE_CORES is not None:
            kwargs["trace_cores"] = TRACE_CORES
    res = None
    for attempt in range(3):
        try:
            res = bass_utils.run_bass_kernel_spmd(
                nc, in_maps, core_ids=list(range(E)), **kwargs)
            break
        except Exception:
            if attempt == 2:
                raise
            import time
            time.sleep(15)
    LAST_RESULT = res

    mean = np.empty((nb, A), np.float32)
    log_std = np.empty((nb, A), np.float32)
    for e in range(E):
        out = np.asarray(res.results[e]["out"])          # [34, npad] bf16
        ofull = out.T.astype(np.float32)
        idx = idx_per_e[e]
        mean[idx] = ofull[:len(idx), :A] + bm[e]
        log_std[idx] = 3.5 * np.tanh(ofull[:len(idx), A:AOUT] + bs[e]) - 1.5
    return mean, log_std
